# revision 1
# baseline (speedup 1.0000x reference)
"""Trainium2 Bass kernel for a single attention head.

Reference computation (fp32):
    K = Xk @ WK ; V = Xv @ WV ; Q = Xq @ WQ          # [B,S,D] @ [D,E]
    scores = Q @ K^T / sqrt(S)                        # [B,S,S]  (scale = sqrt(seq_len)!)
    out = softmax(scores, axis=-1) @ V                # [B,S,E]

Shapes: B=4, S=2048, D=1024, E=1024.

Sharding: 8 cores = (batch b, query-half h).  Each core handles all S=2048
keys/values of its batch (K/V projections duplicated between the two cores
of a batch) and QH=1024 queries.

Per-core device algorithm (everything f32, matmuls in float32r):
  Inputs are fed pre-transposed ([D, s] layout) so the contraction dim (d)
  lands on SBUF partitions:
    Q^T[e,q]   = sum_d WQ[d,e]-stationary x XqT[d,q]          (resident SBUF)
    K^T[e,s]   = sum_d WK[d,e]-stationary x XkT[d,s] -> DRAM scratch
                 (round-trips; SBUF cannot hold K^T + V + P^T at once)
    V[s,e]     = sum_d XvT[d,s]-stationary x WV[d,e]          (resident SBUF)
    S^T[k,q]   = sum_e K^T-tile-stationary x Q^T     (psum)
    P^T[k,q]   = exp(S^T / sqrt(2048))               (no max-subtraction:
                 |scores| is bounded ~35 here, exp stays finite in fp32)
    O[q,e]     = sum_k P^T-tile-stationary x V[k,e]  (psum accumulate)
    den[q]     = sum_k P^T-tile-stationary x ones    (matmul with ones col)
    out[q,e]   = O[q,e] / den[q]
"""

import numpy as np

import concourse.bass as bass
import concourse.tile as tile
from concourse import bacc, mybir
from concourse.bass_utils import run_bass_kernel_spmd

F32 = mybir.dt.float32
F32R = mybir.dt.float32r

B, S, D, E = 4, 2048, 1024, 1024
QH = S // 2          # queries per core
N_CORES = 8


def _build(nc, D, S, E, QH, reps=1):
    """Emit the Tile program. All dims divisible by 128."""
    P = 128
    DT, ET, NKT = D // P, E // P, S // P        # d-, e-, k-tile counts
    CW = min(512, S)                             # s-dim moving chunk
    QCW = min(512, QH)                           # q-dim moving chunk
    NSC, NQC = S // CW, QH // QCW
    NQT = QH // P                                # q-tiles
    EC = min(512, E)                             # e-dim chunk
    NEC = E // EC
    scale = 1.0 / float(np.sqrt(np.float32(S)))

    xq_d = nc.dram_tensor("xqT", [D, QH], F32R, kind="ExternalInput").ap()
    xk_d = nc.dram_tensor("xkT", [D, S], F32R, kind="ExternalInput").ap()
    xv_d = nc.dram_tensor("xvT", [D, S], F32R, kind="ExternalInput").ap()
    wq_d = nc.dram_tensor("wq", [D, E], F32R, kind="ExternalInput").ap()
    wk_d = nc.dram_tensor("wk", [D, E], F32R, kind="ExternalInput").ap()
    wv_d = nc.dram_tensor("wv", [D, E], F32R, kind="ExternalInput").ap()
    o_d = nc.dram_tensor("o", [QH, E], F32, kind="ExternalOutput").ap()
    on_d = nc.dram_tensor("onesc", [128, 2], F32R, kind="ExternalInput").ap()
    ktr_d = nc.dram_tensor("ktr", [ET, P, S], F32R).ap()   # K^T scratch

    with tile.TileContext(nc) as tc:
      for _rep in range(reps):
        with tc.tile_pool(name="singles", bufs=1) as singles:
            qt_sb = singles.tile([P, ET, QH], F32R)     # Q^T resident
            v_sb = singles.tile([P, NKT, E], F32R)      # V resident
            ones = singles.tile([P, 2], F32R)
            nc.scalar.dma_start(out=ones, in_=on_d)
            ktt0 = singles.tile([P, ET, P], F32R)

            with tc.tile_pool(name="wpool", bufs=3) as wpool, \
                 tc.tile_pool(name="xpool", bufs=3) as xpool, \
                 tc.tile_pool(name="cpout", bufs=3) as cpout, \
                 tc.tile_pool(name="ps1", bufs=8, space="PSUM") as ps1:
                _projections(nc, wpool, xpool, cpout, ps1, qt_sb, v_sb,
                             xq_d, xk_d, xv_d, wq_d, wk_d, wv_d, ktr_d,
                             P, DT, ET, CW, QCW, NSC, NQC, EC, NEC)

            # prefetch the first scores K-tile into a dedicated SBUF tile:
            # emitted after the K^T writes (RAW dep), address-disjoint from
            # the phase-2 pools, so it loads during the V projection.
            nc.scalar.dma_start(
                out=ktt0,
                in_=ktr_d[:, :, 0:P].rearrange("t p k -> p t k"))

            with tc.tile_pool(name="ptpool", bufs=1) as ptpool, \
                 tc.tile_pool(name="ktpool", bufs=4) as ktpool, \
                 tc.tile_pool(name="opool", bufs=3) as opool, \
                 tc.tile_pool(name="rpool", bufs=4) as rpool:
                pt_sb = ptpool.tile([P, NKT, QH], F32R)  # P^T = exp(scores^T)

                with tc.tile_pool(name="ps_sc", bufs=4, space="PSUM") as psc:
                    _scores_exp(nc, psc, ktpool, qt_sb, pt_sb, ktr_d, ktt0,
                                P, ET, NKT, QCW, NQC, QH, scale)

                with tc.tile_pool(name="ps_o", bufs=3, space="PSUM") as pso, \
                     tc.tile_pool(name="ps_den", bufs=2, space="PSUM") as psd:
                    _pv(nc, pso, psd, opool, rpool, pt_sb, v_sb, ones, o_d,
                        P, NQT, NKT, EC, NEC, E)
    return nc


def _projections(nc, wpool, xpool, cpout, ps1, qt_sb, v_sb,
                 xq_d, xk_d, xv_d, wq_d, wk_d, wv_d, ktr_d,
                 P, DT, ET, CW, QCW, NSC, NQC, EC, NEC):
    # --- Q^T = sum_d WQ[d,e](stationary) x XqT[d,q] -> resident SBUF ---
    wq_halves = []
    for _h in range(2):
        _wt = wpool.tile([P, DT // 2, ET * P], F32R, tag="w",
                         name="wqh%d" % _h)
        for _dt in range(DT // 2):
            _gdt = _h * (DT // 2) + _dt
            nc.scalar.dma_start(out=_wt[:, _dt, :],
                                in_=wq_d[_gdt * P:(_gdt + 1) * P, :])
        wq_halves.append(_wt)
    # dt-outer so the very first matmuls consume W/X chunks in DMA-arrival
    # order (kernel start is serial-DMA-paced; stationary reuse matters less
    # here than arrival order).
    for sc in range(NQC):
        xq_sb = xpool.tile([P, DT, QCW], F32R, tag="x")
        for _dt in range(DT):
            nc.sync.dma_start(
                out=xq_sb[:, _dt, :],
                in_=xq_d[_dt * P:(_dt + 1) * P, sc * QCW:(sc + 1) * QCW])
        pss = [ps1.tile([P, QCW], F32, tag="ps", name=f"psq{et}")
               for et in range(ET)]
        for dt_ in range(DT):
            x_sl = xq_sb[:, dt_, :]
            for et in range(ET):
                w_sl = wq_halves[dt_ // (DT // 2)][
                    :, dt_ % (DT // 2), et * P:(et + 1) * P]
                nc.tensor.matmul(
                    pss[et], w_sl, x_sl,
                    start=(dt_ == 0), stop=(dt_ == DT - 1),
                    skip_group_check=True)
        for et in range(ET):
            nc.scalar.copy(
                out=qt_sb[:, et, sc * QCW:(sc + 1) * QCW], in_=pss[et])

    # --- K^T = sum_d WK[d,e](stationary) x XkT[d,s] -> DRAM scratch ---
    wk_halves = []
    for _h in range(2):
        _wt = wpool.tile([P, DT // 2, ET * P], F32R, tag="w",
                         name="wkh%d" % _h)
        for _dt in range(DT // 2):
            _gdt = _h * (DT // 2) + _dt
            nc.scalar.dma_start(out=_wt[:, _dt, :],
                                in_=wk_d[_gdt * P:(_gdt + 1) * P, :])
        wk_halves.append(_wt)
    for scp in range(0, NSC, 2):
        pair = [sc for sc in (scp, scp + 1) if sc < NSC]
        xks = []
        for sc in pair:
            xk_sb = xpool.tile([P, DT, CW], F32R, tag="x")
            for _dt in range(DT):
                nc.sync.dma_start(
                    out=xk_sb[:, _dt, :],
                    in_=xk_d[_dt * P:(_dt + 1) * P, sc * CW:(sc + 1) * CW])
            xks.append(xk_sb)
        for et in range(ET):
            pss = [ps1.tile([P, CW], F32, tag="ps", name=f"psk{i}")
                   for i in range(len(pair))]
            for dt_ in range(DT):
                w_sl = wk_halves[dt_ // (DT // 2)][:, dt_ % (DT // 2), et * P:(et + 1) * P]
                for i in range(len(pair)):
                    nc.tensor.matmul(
                        pss[i], w_sl, xks[i][:, dt_, :],
                        start=(dt_ == 0), stop=(dt_ == DT - 1),
                        skip_group_check=True)
            for i, sc in enumerate(pair):
                kt_out = cpout.tile([P, CW], F32R, tag="c")
                nc.scalar.copy(out=kt_out, in_=pss[i])
                nc.sync.dma_start(
                    out=ktr_d[et, :, sc * CW:(sc + 1) * CW], in_=kt_out)

    # --- V = sum_d XvT[d,s](stationary) x WV[d,e] -> resident SBUF ---
    wv_halves = []
    for _h in range(2):
        _wt = wpool.tile([P, DT // 2, ET * P], F32R, tag="w",
                         name="wvh%d" % _h)
        for _dt in range(DT // 2):
            _gdt = _h * (DT // 2) + _dt
            nc.scalar.dma_start(out=_wt[:, _dt, :],
                                in_=wv_d[_gdt * P:(_gdt + 1) * P, :])
        wv_halves.append(_wt)
    for sc in range(NSC):
        xv_sb = xpool.tile([P, DT, CW], F32R, tag="x")
        for _dt in range(DT):
            nc.sync.dma_start(
                out=xv_sb[:, _dt, :],
                in_=xv_d[_dt * P:(_dt + 1) * P, sc * CW:(sc + 1) * CW])
        for stl in range(CW // P):               # s-tiles inside chunk
            st = sc * (CW // P) + stl
            pss = [ps1.tile([P, EC], F32, tag="ps", name=f"psv{ec}")
                   for ec in range(NEC)]
            for dt_ in range(DT):
                x_sl = xv_sb[:, dt_, stl * P:(stl + 1) * P]
                for ec in range(NEC):
                    nc.tensor.matmul(
                        pss[ec], x_sl,
                        wv_halves[dt_ // (DT // 2)][:, dt_ % (DT // 2), ec * EC:(ec + 1) * EC],
                        start=(dt_ == 0), stop=(dt_ == DT - 1),
                        skip_group_check=True)
            for ec in range(NEC):
                nc.vector.tensor_copy(
                    out=v_sb[:, st, ec * EC:(ec + 1) * EC], in_=pss[ec])


def _scores_exp(nc, psc, ktpool, qt_sb, pt_sb, ktr_d, ktt0,
                P, ET, NKT, QCW, NQC, QH, scale):
    for kt in range(NKT):
        if kt == 0:
            ktt = ktt0
        else:
            ktt = ktpool.tile([P, ET, P], F32R, tag="kt")
            nc.scalar.dma_start(
                out=ktt,
                in_=ktr_d[:, :, kt * P:(kt + 1) * P].rearrange(
                    "t p k -> p t k"))
        ps_sc = psc.tile([P, QH], F32, tag="sc")
        for et in range(ET):
            kt_sl = ktt[:, et, :]
            for qc in range(NQC):
                qsl = slice(qc * QCW, (qc + 1) * QCW)
                nc.tensor.matmul(
                    ps_sc[:, qsl], kt_sl, qt_sb[:, et, qsl],
                    start=(et == 0), stop=(et == ET - 1),
                    skip_group_check=True)
        nc.scalar.activation(
            out=pt_sb[:, kt, :], in_=ps_sc,
            func=mybir.ActivationFunctionType.Exp, scale=scale)


def _pv(nc, pso, psd, opool, rpool, pt_sb, v_sb, ones, o_d,
        P, NQT, NKT, EC, NEC, E):
    for qt in range(NQT):
        qsl = slice(qt * P, (qt + 1) * P)
        ps_o = pso.tile([P, E], F32, tag="o")
        ps_den = psd.tile([P, 2], F32, tag="den")
        for kt in range(NKT):
            pt_sl = pt_sb[:, kt, qsl]
            nc.tensor.matmul(
                ps_den, pt_sl, ones,
                start=(kt == 0), stop=(kt == NKT - 1),
                skip_group_check=True)
            for ec in range(NEC):
                esl = slice(ec * EC, (ec + 1) * EC)
                nc.tensor.matmul(
                    ps_o[:, esl], pt_sl, v_sb[:, kt, esl],
                    start=(kt == 0), stop=(kt == NKT - 1),
                    skip_group_check=True)
        recip = rpool.tile([P, 1], F32, tag="r")
        nc.vector.reciprocal(out=recip, in_=ps_den[:, 0:1])
        o_sb = opool.tile([P, E], F32, tag="ob")
        if qt % 2 == 0:
            nc.vector.tensor_scalar_mul(o_sb, ps_o, recip)
        else:
            nc.scalar.activation(
                out=o_sb, in_=ps_o,
                func=mybir.ActivationFunctionType.Copy, scale=recip)
        nc.sync.dma_start(out=o_d[qsl, :], in_=o_sb)


_ONES = np.ones((128, 2), dtype=np.float32)

_CACHE = {}


def _get_nc(dims):
    if dims not in _CACHE:
        nc = bacc.Bacc("TRN2", target_bir_lowering=False, debug=False)
        _build(nc, *dims)
        nc.compile()
        _CACHE[dims] = nc
    return _CACHE[dims]


def kernel(inputs_for_keys, inputs_for_values, inputs_for_queries, WK, WV, WQ):
    xk = np.asarray(inputs_for_keys, dtype=np.float32)
    xv = np.asarray(inputs_for_values, dtype=np.float32)
    xq = np.asarray(inputs_for_queries, dtype=np.float32)
    wk = np.ascontiguousarray(np.asarray(WK, dtype=np.float32))
    wv = np.ascontiguousarray(np.asarray(WV, dtype=np.float32))
    wq = np.ascontiguousarray(np.asarray(WQ, dtype=np.float32))

    nc = _get_nc((D, S, E, QH))

    in_maps = []
    for c in range(N_CORES):
        b, h = c // 2, c % 2
        in_maps.append({
            "xkT": np.ascontiguousarray(xk[b].T),
            "xvT": np.ascontiguousarray(xv[b].T),
            "xqT": np.ascontiguousarray(xq[b, h * QH:(h + 1) * QH, :].T),
            "wk": wk, "wv": wv, "wq": wq,
            "onesc": _ONES,
        })

    results = run_bass_kernel_spmd(nc, in_maps, list(range(N_CORES))).results

    out = np.empty((B, S, E), dtype=np.float32)
    for c in range(N_CORES):
        b, h = c // 2, c % 2
        out[b, h * QH:(h + 1) * QH, :] = results[c]["o"]
    return out

